# revision 1
# baseline (speedup 1.0000x reference)
"""Trainium2 Bass kernel for nn_AudioLSTM (2-layer LSTM + 2-layer FC head).

Latency-optimized per-step dependency cycle (~2.0us/step, was ~3.0us).

Strategy (per core; pure data parallelism over batch, 8 cores x 64 batch):
  - Keep all recurrent state in SBUF; one fused loop over T=1000 steps,
    two batch halves of 32 as independent latency-hiding pipelines.
  - State tile st [97, 32] bf16 per half: [H1(64); H2(32); ones] where
    H = 2*h (scale absorbed into packed weights).  LSTM2 runs one step
    behind LSTM1 so both layers share one state/matmul/activation set.
  - Gate matmuls split into an x-side (K=26, weights wx, start=True) that
    is PREFETCHED into the psum bank a step ahead, and an H-side (K=97,
    start=False accumulate) that is the only H-dependent work, so the
    recurrence's matmul phase starts ~50ns after the H update lands.
    PSUM zero-region semantics: start=True marks the WHOLE 2KB region
    pending-zero, so only the FIRST x-side matmul sets start and only the
    LAST H-side matmul sets stop.
  - tanh-everywhere: sigma(z) = (1+tanh(z/2))/2; the 1/2 scales are folded
    into the packed weights, so ONE Tanh activation covers all 4 gates
    (gate column order i,f,o,g).
  - mega tile [96, 5*bh] holds the gate tanh area and the cell state C=2*c
    in the 5th slot, adjacent to the g-gate, so ONE DVE STT computes
    [Bv|Av] = ([ti|tf]+1) * [tg|C].  Then C'=0.5*Av+Bv [DVE],
    th=tanh(0.5*C') [Act], H=(to+1)*th -> st [DVE].
  - Iteration 0 uses a weight copy with the LSTM2 columns zeroed, which makes
    the one-step-behind LSTM2 start exactly from h2=c2=0.
"""
import os
import sys
from contextlib import ExitStack

import numpy as np

sys.path.insert(0, "/opt/trn_rl_repo")

import ml_dtypes

import concourse.bacc as bacc
import concourse.mybir as mybir
from concourse import bass_utils, tile

AF = mybir.ActivationFunctionType
ALU = mybir.AluOpType
BF16 = mybir.dt.bfloat16
F32 = mybir.dt.float32

IN, H1, H2, F1, OUT = 26, 64, 32, 16, 10
B, T = 512, 1000
NCORES = 8
BL = B // NCORES          # 64 batch per core
NH = 2                    # batch halves per core (latency pipelining)
TC = 250                  # time chunk for x staging
KP = 97                   # state rows: 64 H1 + 32 H2 + 1 ones (bias)


def _build_body(ctx: ExitStack, tc_: tile.TileContext, x, w, w0, wx, wfc1, wfc2,
                out, nh=NH, bv_eng="dve"):
    nc = tc_.nc
    bh = BL // nh

    const = ctx.enter_context(tc_.tile_pool(name="const", bufs=1))
    xpool = ctx.enter_context(tc_.tile_pool(name="xp", bufs=2))
    psum = ctx.enter_context(tc_.tile_pool(name="ps", bufs=3, space="PSUM"))
    work = ctx.enter_context(tc_.tile_pool(name="wk", bufs=4))

    w_sb = const.tile([KP, 384], BF16)
    nc.sync.dma_start(out=w_sb, in_=w)
    w0_sb = const.tile([KP, 384], BF16)
    nc.sync.dma_start(out=w0_sb, in_=w0)
    wx_sb = const.tile([KP, 384], BF16)
    nc.sync.dma_start(out=wx_sb, in_=wx)
    wfc1_sb = const.tile([33, F1], BF16)
    nc.sync.dma_start(out=wfc1_sb, in_=wfc1)
    wfc2_sb = const.tile([33, OUT], BF16)
    nc.sync.dma_start(out=wfc2_sb, in_=wfc2)

    # mega tile per half: cols 0:4bh = gate tanh area (i,f,o,g), 4bh:5bh = C
    # (C adjacent to g so one STT computes [Bv|Av] from [ti|tf] and [tg|C]).
    sts = []
    megas = []
    for h in range(nh):
        st_h = const.tile([KP, bh], BF16, name=f"st{h}")
        nc.any.memset(st_h[0:96, :], 0.0)
        nc.any.memset(st_h[96:97, :], 1.0)
        mega_h = const.tile([96, 5 * bh], F32, name=f"mega{h}")
        nc.any.memset(mega_h, 0.0)
        sts.append(st_h)
        megas.append(mega_h)

    out_sb = const.tile([OUT, BL], F32)

    # x DRAM view: [BL, IN, T] -> chunk [IN, BL, width].  The first chunk is
    # small so its DMA (~26us for a full 250-step chunk; descriptor-bound)
    # doesn't gate pipeline startup; later chunks' DMAs are hoisted ~250
    # steps early by the scheduler (no WAR constraint) and fully hidden.
    bounds = [0, 32] + list(range(32 + TC, T, TC)) + [T]
    cstart = {bounds[i]: i for i in range(len(bounds) - 1)}
    tt_of = {}
    for ci in range(len(bounds) - 1):
        for k in range(bounds[ci], bounds[ci + 1]):
            tt_of[k] = k - bounds[ci]
    xk3 = None

    def load_chunk(ci):
        nonlocal xk3
        width = bounds[ci + 1] - bounds[ci]
        xk = xpool.tile([IN, width * BL], BF16, name="xk", tag="xk")
        xk3 = xk.rearrange("p (t b) -> p t b", b=BL)
        xin = x[:, bounds[ci]:bounds[ci + 1], :]
        # x is pre-transposed to [IN, T, BL] bf16 on the host, so this DMA is
        # ~26 contiguous descriptors instead of 26*64 scatter descriptors.
        nc.gpsimd.dma_start(out=xk3, in_=xin)

    load_chunk(0)

    # x-side gate matmuls for step k: prefetched into the psum bank a step
    # ahead (start=True); the H-side matmuls accumulate on top (stop=True).
    # x_t is staged into a contiguous K=97-padded tile (rows 26:97 zero) so
    # every matmul uses the identical (128,128) PE tile config.
    xts = []
    for h in range(nh):
        pair = []
        for j in range(2):
            xt_hj = const.tile([KP, bh], BF16, name=f"xt{h}_{j}")
            nc.any.memset(xt_hj, 0.0)
            pair.append(xt_hj)
        xts.append(pair)

    pss = [None] * nh

    def xmm(k, h):
        tt = tt_of[k]
        xt = xts[h][k % 2]
        nc.gpsimd.tensor_copy(out=xt[0:IN, :], in_=xk3[:, tt, h * bh:(h + 1) * bh])
        ps = psum.tile([96, 4 * bh], F32, name="ps", tag=f"ps{h}")
        for gi in range(4):
            # start=True ONLY on gi==0: start marks the whole 2KB psum
            # zero-region pending-zero; re-marking on later gates would make
            # the H-side matmuls overwrite (not accumulate) gates 0..2.
            nc.tensor.matmul(
                ps[:, gi * bh:(gi + 1) * bh],
                wx_sb[:, gi * 96:(gi + 1) * 96],
                xt,
                start=(gi == 0),
                stop=False,
            )
        pss[h] = ps

    for h in range(nh):
        xmm(0, h)

    for k in range(T + 1):
        wsel = w0_sb if k == 0 else w_sb
        last = k == T
        for h in range(nh):
            st_h, mega = sts[h], megas[h]
            AS = mega[:, 0:4 * bh]
            Cc = mega[:, 4 * bh:5 * bh]
            # --- PE: 4 H-side gate matmuls (accumulate onto x-side) ---
            if last:
                ps = psum.tile([96, 4 * bh], F32, name="ps", tag=f"ps{h}")
                pss[h] = ps
            else:
                ps = pss[h]
            for gi in range(4):
                nc.tensor.matmul(
                    ps[:, gi * bh:(gi + 1) * bh],
                    wsel[:, gi * 96:(gi + 1) * 96],
                    st_h,
                    start=(last and gi == 0),
                    stop=(gi == 3),
                )
            # --- Act: gate tanh (i,f,o,g) ---
            nc.scalar.activation(AS, ps, AF.Tanh)
            # --- DVE: [Bv|Av] = ([ti|tf] + 1) * [tg|C] in ONE op ---
            BA = work.tile([96, 2 * bh], F32, name="BA", tag=f"BA{h}")
            nc.vector.scalar_tensor_tensor(
                BA, AS[:, 0:2 * bh], 1.0, mega[:, 3 * bh:5 * bh],
                ALU.add, ALU.mult
            )
            # --- DVE: C = 0.5*Av + Bv ---
            nc.vector.scalar_tensor_tensor(
                Cc, BA[:, bh:2 * bh], 0.5, BA[:, 0:bh], ALU.mult, ALU.add
            )
            # --- Act: th = tanh(0.5*C) ---
            th = work.tile([96, bh], F32, name="th", tag=f"th{h}")
            nc.scalar.activation(th, Cc, AF.Tanh, scale=0.5)
            # --- DVE: H = (to+1)*th -> st rows 0:96 ---
            nc.vector.scalar_tensor_tensor(
                st_h[0:96, :], AS[:, 2 * bh:3 * bh], 1.0, th,
                ALU.add, ALU.mult
            )
            # --- PE: prefetch x-side matmuls for step k+1 ---
            if k + 1 < T:
                if h == 0 and (k + 1) in cstart:
                    load_chunk(cstart[k + 1])
                xmm(k + 1, h)

    # FC head: H2-state (2*h2) lives in st rows 64:96, ones row at 122
    for h in range(nh):
        st_h = sts[h]
        fcin = work.tile([33, bh], BF16, name="fcin", tag="fcin", bufs=2)
        nc.vector.tensor_copy(out=fcin[0:32, :], in_=st_h[64:96, :])
        nc.any.memset(fcin[32:33, :], 1.0)
        fps = psum.tile([F1, bh], F32, name="fps", tag="fps", bufs=1)
        nc.tensor.matmul(fps, wfc1_sb, fcin, start=True, stop=True)
        rr = work.tile([33, bh], BF16, name="rr", tag="rr")
        nc.any.memset(rr[0:33, :], 0.0)
        nc.any.memset(rr[32:33, :], 1.0)
        nc.scalar.activation(rr[0:F1, :], fps, AF.Relu)
        ops = psum.tile([OUT, bh], F32, name="ops", tag="ops", bufs=1)
        nc.tensor.matmul(ops, wfc2_sb, rr, start=True, stop=True)
        nc.vector.tensor_copy(out=out_sb[:, h * bh:(h + 1) * bh], in_=ops)
    nc.sync.dma_start(out=out, in_=out_sb)


def build_program(nh=NH, bv_eng="pool"):
    nc = bacc.Bacc(
        "TRN2",
        target_bir_lowering=False,
        debug=False,
        num_devices=NCORES,
    )
    x_d = nc.dram_tensor("x", [IN, T, BL], BF16, kind="ExternalInput")
    w_d = nc.dram_tensor("w", [KP, 384], BF16, kind="ExternalInput")
    w0_d = nc.dram_tensor("w0", [KP, 384], BF16, kind="ExternalInput")
    wx_d = nc.dram_tensor("wx", [KP, 384], BF16, kind="ExternalInput")
    wfc1_d = nc.dram_tensor("wfc1", [33, F1], BF16, kind="ExternalInput")
    wfc2_d = nc.dram_tensor("wfc2", [33, OUT], BF16, kind="ExternalInput")
    out_d = nc.dram_tensor("out", [OUT, BL], F32, kind="ExternalOutput")

    with tile.TileContext(nc) as tc_, ExitStack() as ctx:
        _build_body(
            ctx, tc_, x_d.ap(), w_d.ap(), w0_d.ap(), wx_d.ap(), wfc1_d.ap(),
            wfc2_d.ap(), out_d.ap(), nh=nh, bv_eng=bv_eng,
        )
    nc.compile()
    return nc


def pack_weights(inp):
    """Pack LSTM+FC weights into the fused bf16 layout (see module docstring)."""
    s = {"i": 0.5, "f": 0.5, "o": 0.5, "g": 1.0}

    def rows(q, H):
        idx = {"i": 0, "f": 1, "g": 2, "o": 3}[q]  # pytorch gate order
        return slice(idx * H, (idx + 1) * H)

    # st rows: 0:64 H1-state (2*h1), 64:96 H2-state (2*h2), 96 ones (bias)
    W = np.zeros((KP, 384), np.float32)
    Wx = np.zeros((KP, 384), np.float32)
    for gi, q in enumerate(["i", "f", "o", "g"]):
        c0 = gi * 96
        r1 = rows(q, H1)
        Wx[0:IN, c0:c0 + 64] = s[q] * inp["w_ih1"][r1].T
        W[96, c0:c0 + 64] = s[q] * (inp["b_ih1"][r1] + inp["b_hh1"][r1])
        W[0:64, c0:c0 + 64] = s[q] * 0.5 * inp["w_hh1"][r1].T
        r2 = rows(q, H2)
        W[0:64, c0 + 64:c0 + 96] = s[q] * 0.5 * inp["w_ih2"][r2].T
        W[64:96, c0 + 64:c0 + 96] = s[q] * 0.5 * inp["w_hh2"][r2].T
        W[96, c0 + 64:c0 + 96] = s[q] * (inp["b_ih2"][r2] + inp["b_hh2"][r2])
    W0 = W.copy()
    for gi in range(4):
        W0[:, gi * 96 + 64:gi * 96 + 96] = 0.0

    fc1 = np.zeros((33, F1), np.float32)
    fc1[0:32] = 0.5 * inp["w_fc1"].T
    fc1[32] = inp["b_fc1"]
    fc2 = np.zeros((33, OUT), np.float32)
    fc2[0:F1] = inp["w_fc2"].T
    fc2[32] = inp["b_fc2"]
    cast = lambda a: a.astype(ml_dtypes.bfloat16)
    return cast(W), cast(W0), cast(Wx), cast(fc1), cast(fc2)


_NC_CACHE = None


def get_program():
    global _NC_CACHE
    if _NC_CACHE is None:
        nh = int(os.environ.get("KERNEL_NH", NH))
        bv_eng = os.environ.get("KERNEL_BV", "pool")
        _NC_CACHE = build_program(nh=nh, bv_eng=bv_eng)
    return _NC_CACHE


def _make_in_maps(inp):
    W, W0, Wx, fc1, fc2 = pack_weights(inp)
    xc = np.ascontiguousarray(inp["x"][:, 0])  # [512, 26, 1000] fp32
    in_maps = []
    for c in range(NCORES):
        in_maps.append({
            "x": np.ascontiguousarray(
                xc[c * BL:(c + 1) * BL].transpose(1, 2, 0)
            ).astype(ml_dtypes.bfloat16),
            "w": W,
            "w0": W0,
            "wx": Wx,
            "wfc1": fc1,
            "wfc2": fc2,
        })
    return in_maps


def kernel(**inputs):
    inp = {k: np.asarray(v) for k, v in inputs.items()}
    in_maps = _make_in_maps(inp)
    nc = get_program()
    res = bass_utils.run_bass_kernel_spmd(nc, in_maps, core_ids=list(range(NCORES)))
    outs = [np.asarray(res.results[c]["out"], np.float32) for c in range(NCORES)]
    return np.concatenate([o.T for o in outs], axis=0).astype(np.float32)


if __name__ == "__main__":
    rng = np.random.default_rng(0)
    fake = {
        "x": rng.standard_normal((B, 1, IN, T), dtype=np.float32),
        "w_ih1": rng.standard_normal((4 * H1, IN), dtype=np.float32) * 0.1,
        "w_hh1": rng.standard_normal((4 * H1, H1), dtype=np.float32) * 0.1,
        "b_ih1": rng.standard_normal(4 * H1).astype(np.float32) * 0.1,
        "b_hh1": rng.standard_normal(4 * H1).astype(np.float32) * 0.1,
        "w_ih2": rng.standard_normal((4 * H2, H1), dtype=np.float32) * 0.1,
        "w_hh2": rng.standard_normal((4 * H2, H2), dtype=np.float32) * 0.1,
        "b_ih2": rng.standard_normal(4 * H2).astype(np.float32) * 0.1,
        "b_hh2": rng.standard_normal(4 * H2).astype(np.float32) * 0.1,
        "w_fc1": rng.standard_normal((F1, H2), dtype=np.float32) * 0.1,
        "b_fc1": rng.standard_normal(F1).astype(np.float32) * 0.1,
        "w_fc2": rng.standard_normal((OUT, F1), dtype=np.float32) * 0.1,
        "b_fc2": rng.standard_normal(OUT).astype(np.float32) * 0.1,
    }
    y = kernel(**fake)
    print("kernel output", y.shape, y.dtype, np.abs(y).max())

